# revision 1
# baseline (speedup 1.0000x reference)
import sys

sys.path.insert(0, "/opt/trn_rl_repo")
import numpy as np
import concourse.bass as bass
import concourse.mybir as mybir
from concourse import bacc
from concourse.tile import TileContext
from concourse.bass_utils import run_bass_kernel_spmd

C = 192
HEADS = 8
D = C // HEADS  # 24
N = 4096
NT = 8  # n tiles of 512
MB = 32  # m blocks of 128
EPS = 1e-5

_cache = {}


def _build_program():
    if "nc" in _cache:
        return _cache["nc"]
    f32 = mybir.dt.float32
    f32r = mybir.dt.float32r
    nc = bacc.Bacc("TRN2", target_bir_lowering=False, debug=False, num_devices=8)
    q_d = nc.dram_tensor("q", [D, N], f32r, kind="ExternalInput").ap()
    k_d = nc.dram_tensor("k", [D, N], f32r, kind="ExternalInput").ap()
    vt_d = nc.dram_tensor("vt", [128, MB * (D + 1)], f32r, kind="ExternalInput").ap()
    wp_d = nc.dram_tensor("wp", [D + 1, C], f32, kind="ExternalInput").ap()
    tp_d = nc.dram_tensor("tp", [1, 1], f32, kind="ExternalInput").ap()
    y_d = nc.dram_tensor("y", [C, N], f32, kind="ExternalOutput").ap()

    with TileContext(nc) as tc:
        with (
            tc.tile_pool(name="persist", bufs=1) as pp,
            tc.tile_pool(name="sb", bufs=2) as sb,
            tc.tile_pool(name="sp", bufs=1, space="PSUM") as spp,
            tc.tile_pool(name="ac", bufs=2, space="PSUM") as acp,
        ):
            q_s = pp.tile([D, N], f32r, tag="q")
            k_s = pp.tile([D, N], f32r, tag="k")
            vt_s = pp.tile([128, MB * (D + 1)], f32r, tag="vt")
            wp_s = pp.tile([D + 1, C], f32, tag="wp")
            tpb = pp.tile([128, 1], f32, tag="tp")
            ones = pp.tile([1, 32], f32, tag="on")
            y_a = pp.tile([128, N], f32, tag="ya")
            y_b = pp.tile([64, N], f32, tag="yb")
            nc.sync.dma_start(out=q_s[:], in_=q_d[:])
            nc.sync.dma_start(out=k_s[:], in_=k_d[:])
            nc.sync.dma_start(out=vt_s[:], in_=vt_d[:])
            nc.sync.dma_start(out=wp_s[:], in_=wp_d[:])
            nc.sync.dma_start(out=tpb[:], in_=tp_d.to_broadcast([128, 1]))
            nc.vector.memset(ones[:], 1.0)

            for j in range(NT):
                o2 = acp.tile([D + 1, 512], f32, tag="acc")
                qv = q_s[:, j * 512 : (j + 1) * 512]
                for g in range(NT):
                    sp = spp.tile([128, 2048], f32, tag="sp")
                    for i in range(4):
                        m = 4 * g + i
                        nc.tensor.matmul(
                            sp[:, i * 512 : (i + 1) * 512],
                            k_s[:, m * 128 : (m + 1) * 128],
                            qv,
                            start=True,
                            stop=True,
                        )
                    pt = sb.tile([128, 2048], f32r, tag="pt")
                    nc.scalar.activation(
                        pt[:], sp[:], mybir.ActivationFunctionType.Exp,
                        scale=tpb[:, 0:1],
                    )
                    for i in range(4):
                        m = 4 * g + i
                        nc.tensor.matmul(
                            o2[:],
                            vt_s[:, m * (D + 1) : (m + 1) * (D + 1)],
                            pt[:, i * 512 : (i + 1) * 512],
                            start=(m == 0),
                            stop=(m == MB - 1),
                        )
                u = sb.tile([D + 1, 512], f32, tag="u")
                nc.vector.tensor_copy(u[:], o2[:])
                r = sb.tile([1, 512], f32, tag="r")
                nc.vector.reciprocal(r[:], u[0:1, :])
                rb = acp.tile([D + 1, 512], f32, tag="acc")
                nc.tensor.matmul(rb[:], ones[0:1, 0 : D + 1], r[:], start=True, stop=True)
                un = sb.tile([D + 1, 512], f32, tag="un")
                nc.vector.tensor_mul(un[:], u[:], rb[:])
                ya_ps = acp.tile([128, 512], f32, tag="acc")
                nc.tensor.matmul(ya_ps[:], wp_s[:, 0:128], un[:], start=True, stop=True)
                nc.vector.tensor_copy(y_a[:, j * 512 : (j + 1) * 512], ya_ps[:])
                yb_ps = acp.tile([64, 512], f32, tag="acc")
                nc.tensor.matmul(yb_ps[:], wp_s[:, 128:192], un[:], start=True, stop=True)
                nc.vector.tensor_copy(y_b[:, j * 512 : (j + 1) * 512], yb_ps[:])
            nc.sync.dma_start(out=y_d[0:128, :], in_=y_a[:])
            nc.sync.dma_start(out=y_d[128:192, :], in_=y_b[:])
    nc.compile()
    _cache["nc"] = nc
    return nc


def _front(x, gamma, beta, w_qkv, w_dw, b_dw):
    # LayerNorm over channels + 1x1 conv + depthwise 3x3 conv (host, cheap)
    xf = x.reshape(C, N).astype(np.float64)
    mean = xf.mean(axis=0)
    var = xf.var(axis=0)
    xln = (xf - mean) / np.sqrt(var + EPS)
    xln = xln * gamma[:, None].astype(np.float64) + beta[:, None].astype(np.float64)
    qkv = (w_qkv.astype(np.float64) @ xln).reshape(3 * C, 64, 64)
    pad = np.zeros((3 * C, 66, 66))
    pad[:, 1:65, 1:65] = qkv
    out = np.zeros((3 * C, 64, 64))
    for dy in range(3):
        for dx in range(3):
            out += w_dw[:, 0, dy, dx][:, None, None] * pad[:, dy : dy + 64, dx : dx + 64]
    out += b_dw[:, None, None].astype(np.float64)
    return out.reshape(3 * C, N).astype(np.float32)


def kernel(x, gamma, beta, w_qkv, w_dw, b_dw, w_proj, temperature):
    x = np.asarray(x, dtype=np.float32)
    gamma = np.asarray(gamma, np.float32)
    beta = np.asarray(beta, np.float32)
    w_qkv = np.asarray(w_qkv, np.float32)
    w_dw = np.asarray(w_dw, np.float32)
    b_dw = np.asarray(b_dw, np.float32)
    w_proj = np.asarray(w_proj, np.float32)
    temperature = np.asarray(temperature, np.float32)

    qkv = _front(x, gamma, beta, w_qkv, w_dw, b_dw)
    q, k, v = qkv[0:C], qkv[C : 2 * C], qkv[2 * C :]

    in_maps = []
    for h in range(HEADS):
        sl = slice(h * D, (h + 1) * D)
        vt = np.zeros((128, MB * (D + 1)), np.float32)
        vh = v[sl]
        for m in range(MB):
            vt[:, m * (D + 1)] = 1.0
            vt[:, m * (D + 1) + 1 : (m + 1) * (D + 1)] = vh[:, m * 128 : (m + 1) * 128].T
        wp = np.zeros((D + 1, C), np.float32)
        wp[1:, :] = w_proj[:, sl].T
        in_maps.append(
            {
                "q": np.ascontiguousarray(q[sl]),
                "k": np.ascontiguousarray(k[sl]),
                "vt": vt,
                "wp": wp,
                "tp": temperature.reshape(HEADS, 1)[h : h + 1].reshape(1, 1),
            }
        )

    nc = _build_program()
    res = run_bass_kernel_spmd(nc, in_maps, list(range(8)))
    y = x.reshape(C, N).astype(np.float64).copy()
    for h in range(HEADS):
        y += res.results[h]["y"].astype(np.float64)
    return y.astype(np.float32).reshape(1, C, 64, 64)



# revision 22
# speedup vs baseline: 12.1047x; 12.1047x over previous
import sys

sys.path.insert(0, "/opt/trn_rl_repo")
import numpy as np
import concourse.bass as bass
import concourse.mybir as mybir
from concourse import bacc
from concourse.tile import TileContext

C = 192
HEADS = 8
D = C // HEADS  # 24
N = 4096
NP = N // HEADS  # 512 pixels per core
NT = 8  # n tiles of 512
MB = 32  # m blocks of 128
EPS = 1e-5
TAPS = [(dy, dx) for dy in (-1, 0, 1) for dx in (-1, 0, 1)]
CENTER = TAPS.index((0, 0))

f32 = mybir.dt.float32
f32r = mybir.dt.float32r
f16 = mybir.dt.float16

_cache = {}


def _build_program():
    nc = bacc.Bacc("TRN2", target_bir_lowering=False, debug=False, num_devices=8)
    x_d = nc.dram_tensor("x", [C, NP], f16, kind="ExternalInput").ap()
    # fused (1x1 conv) x (depthwise 3x3): per section s (q/k/v), per tap t,
    # lhsT[c, o] = w_qkv[sec_o, c] * w_dw[sec_o, tap]
    wq_d = nc.dram_tensor("wq", [C, 27 * D], f32, kind="ExternalInput").ap()
    dw_d = nc.dram_tensor("dw", [D, 3], f32, kind="ExternalInput").ap()  # biases
    wp_d = nc.dram_tensor("wp", [D + 1, C], f32, kind="ExternalInput").ap()
    gb_d = nc.dram_tensor("gb", [C, 2], f32, kind="ExternalInput").ap()
    tp_d = nc.dram_tensor("tp", [1, 1], f32, kind="ExternalInput").ap()
    id_d = nc.dram_tensor("id24", [D, D], f32, kind="ExternalInput").ap()
    y_d = nc.dram_tensor("y", [C, NP], f16, kind="ExternalOutput").ap()

    RG = [list(range(8))]

    with TileContext(nc) as tc:
        with (
            tc.tile_pool(name="persist", bufs=1) as pp,
            tc.tile_pool(name="fb", bufs=1) as fb,
            tc.tile_pool(name="sb", bufs=2) as sb,
            tc.tile_pool(name="fp", bufs=2, space="PSUM") as fpp,
            tc.tile_pool(name="sp", bufs=1, space="PSUM") as spp,
            tc.tile_pool(name="ac", bufs=2, space="PSUM") as acp,
            tc.tile_pool(name="dram", bufs=1, space="DRAM") as dp,
        ):
            # ---- persistent sbuf tiles ----
            x16a = pp.tile([128, NP], f16, tag="x16a")
            x16b = pp.tile([64, NP], f16, tag="x16b")
            x32a = pp.tile([128, NP], f32, tag="x32a")
            x32b = pp.tile([64, NP], f32, tag="x32b")
            wq_a = pp.tile([128, 27 * D], f32, tag="wqa")
            wq_b = pp.tile([64, 27 * D], f32, tag="wqb")
            dw_s = pp.tile([D, 3], f32, tag="dw")
            wp_s = pp.tile([D + 1, C], f32, tag="wp")
            gb_a = pp.tile([128, 2], f32, tag="gba")
            gb_b = pp.tile([64, 2], f32, tag="gbb")
            tpb = pp.tile([128, 1], f32, tag="tp")
            id_s = pp.tile([D, D], f32, tag="id")
            ones_c = pp.tile([128, 1], f32, tag="onc")  # lhsT for partition-sum
            ones_r = pp.tile([1, 128], f32, tag="onr")  # lhsT for broadcast
            xf_a = pp.tile([128, N], f32, tag="xfa")  # gathered x_ln rows 0:128
            xf_b = pp.tile([64, N], f32, tag="xfb")  # gathered x_ln rows 128:192
            q_s = pp.tile([D, N], f32r, tag="qs")
            k_s = pp.tile([D, N], f32r, tag="ks")
            v_s = pp.tile([D, N], f32, tag="vs")
            vt_s = pp.tile([128, MB * (D + 1)], f32r, tag="vt")
            y_a = pp.tile([128, N], f32, tag="ya")
            y_b = pp.tile([64, N], f32, tag="yb")
            yr_a = pp.tile([128, NP], f32, tag="yra")
            yr_b = pp.tile([64, NP], f32, tag="yrb")
            y16a = pp.tile([128, NP], f16, tag="y16a")
            y16b = pp.tile([64, NP], f16, tag="y16b")

            # dram bounce buffers for collectives
            ag_in = dp.tile([C, NP], f32, tag="agin")
            ag_out = dp.tile([8 * C, NP], f32, tag="agout")
            rs_in = dp.tile([8 * C, NP], f32, tag="rsin")
            rs_out = dp.tile([C, NP], f32, tag="rsout")

            # ---- load inputs/weights ----
            nc.sync.dma_start(out=x16a[:], in_=x_d[0:128, :])
            nc.sync.dma_start(out=x16b[:], in_=x_d[128:C, :])
            nc.sync.dma_start(out=wq_a[:], in_=wq_d[0:128, :])
            nc.sync.dma_start(out=wq_b[:], in_=wq_d[128:C, :])
            nc.sync.dma_start(out=dw_s[:], in_=dw_d[:])
            nc.sync.dma_start(out=wp_s[:], in_=wp_d[:])
            nc.sync.dma_start(out=gb_a[:], in_=gb_d[0:128, :])
            nc.sync.dma_start(out=gb_b[:], in_=gb_d[128:C, :])
            nc.sync.dma_start(out=tpb[:], in_=tp_d.to_broadcast([128, 1]))
            nc.sync.dma_start(out=id_s[:], in_=id_d[:])
            nc.vector.memset(ones_c[:], 1.0)
            nc.vector.memset(ones_r[:], 1.0)

            nc.vector.tensor_copy(x32a[:], x16a[:])
            nc.vector.tensor_copy(x32b[:], x16b[:])

            # ---- LayerNorm over channel dim (per local pixel) ----
            s1 = fpp.tile([1, NP], f32, tag="fp")
            nc.tensor.matmul(s1[:], ones_c[:, 0:1], x32a[:], start=True, stop=False)
            nc.tensor.matmul(s1[:], ones_c[0:64, 0:1], x32b[:], start=False, stop=True)
            sq_a = fb.tile([128, NP], f32, tag="sq")
            sq_b = fb.tile([64, NP], f32, tag="sqb")
            nc.scalar.square(sq_a[:], x32a[:])
            nc.scalar.square(sq_b[:], x32b[:])
            s2 = fpp.tile([1, NP], f32, tag="fp")
            nc.tensor.matmul(s2[:], ones_c[:, 0:1], sq_a[:], start=True, stop=False)
            nc.tensor.matmul(s2[:], ones_c[0:64, 0:1], sq_b[:], start=False, stop=True)

            mean = fb.tile([1, NP], f32, tag="mean")
            ex2 = fb.tile([1, NP], f32, tag="ex2")
            nc.vector.tensor_scalar_mul(mean[:], s1[:], 1.0 / C)
            nc.vector.tensor_scalar_mul(ex2[:], s2[:], 1.0 / C)
            var = fb.tile([1, NP], f32, tag="var")
            # var = -mean*mean + ex2, then + EPS
            nc.vector.scalar_tensor_tensor(
                var[:], mean[:], -1.0, mean[:],
                op0=mybir.AluOpType.mult, op1=mybir.AluOpType.mult,
            )
            nc.vector.tensor_add(var[:], var[:], ex2[:])
            nc.vector.tensor_scalar_add(var[:], var[:], EPS)
            rcp = fb.tile([1, NP], f32, tag="rcp")
            nc.vector.reciprocal(rcp[:], var[:])
            rstd = fb.tile([1, NP], f32, tag="rstd")
            nc.scalar.sqrt(rstd[:], rcp[:])
            brow = fb.tile([1, NP], f32, tag="brow")
            # brow = -mean * rstd
            nc.vector.scalar_tensor_tensor(
                brow[:], mean[:], -1.0, rstd[:],
                op0=mybir.AluOpType.mult, op1=mybir.AluOpType.mult,
            )
            ab_ps = fpp.tile([128, NP], f32, tag="fp")
            nc.tensor.matmul(ab_ps[:], ones_r[0:1, :], rstd[:], start=True, stop=True)
            bb_ps = fpp.tile([128, NP], f32, tag="fp")
            nc.tensor.matmul(bb_ps[:], ones_r[0:1, :], brow[:], start=True, stop=True)

            xg_a = fb.tile([128, NP], f32, tag="xga")
            xg_b = fb.tile([64, NP], f32, tag="xgb")
            nc.vector.tensor_mul(xg_a[:], x32a[:], ab_ps[:])
            nc.vector.tensor_add(xg_a[:], xg_a[:], bb_ps[:])
            nc.vector.tensor_mul(xg_b[:], x32b[:], ab_ps[0:64, :])
            nc.vector.tensor_add(xg_b[:], xg_b[:], bb_ps[0:64, :])
            # gamma/beta (per-partition scalars); output in f32r for matmuls
            xr_a = fb.tile([128, NP], f32, tag="xra")
            xr_b = fb.tile([64, NP], f32, tag="xrb")
            nc.scalar.activation(
                xr_a[:], xg_a[:], mybir.ActivationFunctionType.Identity,
                bias=gb_a[:, 1:2], scale=gb_a[:, 0:1],
            )
            nc.scalar.activation(
                xr_b[:], xg_b[:], mybir.ActivationFunctionType.Identity,
                bias=gb_b[:, 1:2], scale=gb_b[:, 0:1],
            )

            # ---- AllGather x_ln across cores ----
            nc.gpsimd.dma_start(out=ag_in[0:128, :], in_=xr_a[:])
            nc.gpsimd.dma_start(out=ag_in[128:C, :], in_=xr_b[:])
            nc.gpsimd.collective_compute(
                "AllGather",
                mybir.AluOpType.bypass,
                replica_groups=RG,
                ins=[ag_in.opt()],
                outs=[ag_out.opt()],
            )
            # ag_out rows r*C..(r+1)*C hold core r's x_ln slice (pixels r*NP..)
            ago = ag_out.rearrange("(r c) n -> c r n", r=8)
            nc.sync.dma_start(
                out=xf_a.rearrange("c (r n) -> c r n", r=8),
                in_=ago[0:128],
            )
            nc.sync.dma_start(
                out=xf_b.rearrange("c (r n) -> c r n", r=8),
                in_=ago[128:C],
            )

            # ---- fused (1x1 conv + depthwise 3x3) as 9 shifted matmuls ----
            # qkv_post[o, y, x] = sum_t w_dw[o,t] * sum_c wq[o,c] xln[c,y+dy,x+dx]
            # accumulate all taps in PSUM with clipped (zero-pad) ranges.
            xfa3 = xf_a.rearrange("c (Y X) -> c Y X", X=64)
            xfb3 = xf_b.rearrange("c (Y X) -> c Y X", X=64)
            sec_tiles = (q_s, k_s, v_s)
            for j in range(NT):
                y0 = 8 * j  # first image row of this tile
                for s in range(3):
                    cp = fpp.tile([D, 512], f32, tag="fp")
                    cp3 = cp.rearrange("p (Y X) -> p Y X", X=64)
                    col = (s * 9 + CENTER) * D
                    nc.tensor.matmul(
                        cp[:], wq_a[:, col : col + D],
                        xf_a[:, j * 512 : (j + 1) * 512],
                        start=True, stop=False,
                    )
                    nc.tensor.matmul(
                        cp[:], wq_b[:, col : col + D],
                        xf_b[:, j * 512 : (j + 1) * 512],
                        start=False, stop=False,
                    )
                    for t, (oy, ox) in enumerate(TAPS):
                        if (oy, ox) == (0, 0):
                            continue
                        last = t == len(TAPS) - 1
                        # local row range [ly0, ly1) whose shifted source row
                        # stays inside the image
                        ly0 = max(0, -(y0 + oy))
                        ly1 = min(8, 64 - oy - y0)
                        dx0, dx1 = max(0, -ox), 64 - max(0, ox)
                        col = (s * 9 + t) * D
                        out_ap = cp3[:, ly0:ly1, dx0:dx1]
                        nc.tensor.matmul(
                            out_ap,
                            wq_a[:, col : col + D],
                            xfa3[:, y0 + ly0 + oy : y0 + ly1 + oy, dx0 + ox : dx1 + ox],
                            start=False, stop=False, skip_group_check=True,
                        )
                        nc.tensor.matmul(
                            out_ap,
                            wq_b[:, col : col + D],
                            xfb3[:, y0 + ly0 + oy : y0 + ly1 + oy, dx0 + ox : dx1 + ox],
                            start=False, stop=last, skip_group_check=True,
                        )
                    # bias + copy to sbuf (f32r)
                    nc.scalar.activation(
                        sec_tiles[s][:, j * 512 : (j + 1) * 512], cp[:],
                        mybir.ActivationFunctionType.Identity,
                        bias=dw_s[:, s : s + 1], scale=1.0,
                    )

            # ---- build vt (v transposed blocks with leading ones column) ----
            for i in range(MB):
                nc.scalar.copy(vt_s[:, i * (D + 1) : i * (D + 1) + 1], ones_c[:, 0:1])
            for i in range(MB):
                vp = fpp.tile([128, D], f32, tag="fp")
                nc.tensor.matmul(
                    vp[:],
                    v_s[:, i * 128 : (i + 1) * 128],
                    id_s[:],
                    start=True, stop=True,
                )
                nc.scalar.copy(vt_s[:, i * (D + 1) + 1 : (i + 1) * (D + 1)], vp[:])

            # ---- attention + partial projection ----
            for j in range(NT):
                o2 = acp.tile([D + 1, 512], f32, tag="acc")
                qv = q_s[:, j * 512 : (j + 1) * 512]
                for g in range(NT):
                    sp = spp.tile([128, 2048], f32, tag="sp")
                    for i in range(4):
                        m = 4 * g + i
                        nc.tensor.matmul(
                            sp[:, i * 512 : (i + 1) * 512],
                            k_s[:, m * 128 : (m + 1) * 128],
                            qv,
                            start=True,
                            stop=True,
                        )
                    pt = sb.tile([128, 2048], f32r, tag="pt")
                    nc.scalar.activation(
                        pt[:], sp[:], mybir.ActivationFunctionType.Exp,
                        scale=tpb[:, 0:1],
                    )
                    for i in range(4):
                        m = 4 * g + i
                        nc.tensor.matmul(
                            o2[:],
                            vt_s[:, m * (D + 1) : (m + 1) * (D + 1)],
                            pt[:, i * 512 : (i + 1) * 512],
                            start=(m == 0),
                            stop=(m == MB - 1),
                        )
                u = sb.tile([D + 1, 512], f32, tag="u")
                nc.vector.tensor_copy(u[:], o2[:])
                r = sb.tile([1, 512], f32, tag="r")
                nc.vector.reciprocal(r[:], u[0:1, :])
                rb = acp.tile([D + 1, 512], f32, tag="acc")
                nc.tensor.matmul(
                    rb[:], ones_r[0:1, 0 : D + 1], r[:], start=True, stop=True
                )
                un = sb.tile([D + 1, 512], f32, tag="un")
                nc.vector.tensor_mul(un[:], u[:], rb[:])
                ya_ps = acp.tile([128, 512], f32, tag="acc")
                nc.tensor.matmul(ya_ps[:], wp_s[:, 0:128], un[:], start=True, stop=True)
                nc.vector.tensor_copy(y_a[:, j * 512 : (j + 1) * 512], ya_ps[:])
                yb_ps = acp.tile([64, 512], f32, tag="acc")
                nc.tensor.matmul(yb_ps[:], wp_s[:, 128:C], un[:], start=True, stop=True)
                nc.vector.tensor_copy(y_b[:, j * 512 : (j + 1) * 512], yb_ps[:])

            # ---- ReduceScatter partial projections, add residual, store ----
            rsi = rs_in.rearrange("(r c) n -> c r n", r=8)
            nc.gpsimd.dma_start(out=rsi[0:128], in_=y_a.rearrange("c (r n) -> c r n", r=8))
            nc.gpsimd.dma_start(out=rsi[128:C], in_=y_b.rearrange("c (r n) -> c r n", r=8))
            nc.gpsimd.collective_compute(
                "ReduceScatter",
                mybir.AluOpType.add,
                replica_groups=RG,
                ins=[rs_in.opt()],
                outs=[rs_out.opt()],
            )
            nc.sync.dma_start(out=yr_a[:], in_=rs_out[0:128, :])
            nc.sync.dma_start(out=yr_b[:], in_=rs_out[128:C, :])
            nc.vector.tensor_add(y16a[:], yr_a[:], x32a[:])
            nc.vector.tensor_add(y16b[:], yr_b[:], x32b[:])
            nc.sync.dma_start(out=y_d[0:128, :], in_=y16a[:])
            nc.sync.dma_start(out=y_d[128:C, :], in_=y16b[:])
    nc.compile()
    return nc


def _make_runner():
    """Build the bass program once and a cached jit dispatcher around it,
    mirroring concourse.bass2jax.run_bass_via_pjrt but reusable per call."""
    if "runner" in _cache:
        return _cache["runner"]
    import jax
    import jax.numpy as jnp
    from jax.sharding import Mesh, PartitionSpec as P, NamedSharding
    try:
        from jax import shard_map

        def _shard_map(f, mesh, in_specs, out_specs):
            return shard_map(f, mesh=mesh, in_specs=in_specs, out_specs=out_specs,
                             check_vma=False)
    except ImportError:
        from jax.experimental.shard_map import shard_map

        def _shard_map(f, mesh, in_specs, out_specs):
            return shard_map(f, mesh=mesh, in_specs=in_specs, out_specs=out_specs,
                             check_rep=False)
    from concourse import bass2jax

    nc = _build_program()
    bass2jax.install_neuronx_cc_hook()
    assert nc.dbg_addr is None
    partition_name = nc.partition_id_tensor.name if nc.partition_id_tensor else None

    in_names = []
    out_names = []
    out_avals = []
    zero_shapes = []
    for alloc in nc.m.functions[0].allocations:
        if not isinstance(alloc, mybir.MemoryLocationSet):
            continue
        name = alloc.memorylocations[0].name
        if alloc.kind == "ExternalInput":
            if name != partition_name:
                in_names.append(name)
        elif alloc.kind == "ExternalOutput":
            shape = tuple(alloc.tensor_shape)
            dtype = mybir.dt.np(alloc.dtype)
            out_avals.append(jax.core.ShapedArray(shape, dtype))
            out_names.append(name)
            zero_shapes.append((shape, dtype))
    n_params = len(in_names)
    n_outs = len(out_names)
    all_names = list(in_names) + list(out_names)
    if partition_name is not None:
        all_names.append(partition_name)
    donate = tuple(range(n_params, n_params + n_outs))

    def _body(*args):
        operands = list(args)
        if partition_name is not None:
            operands.append(bass2jax.partition_id_tensor())
        outs = bass2jax._bass_exec_p.bind(
            *operands,
            out_avals=tuple(out_avals),
            in_names=tuple(all_names),
            out_names=tuple(out_names),
            lowering_input_output_aliases=(),
            sim_require_finite=True,
            sim_require_nnan=True,
            nc=nc,
        )
        return tuple(outs)

    devices = jax.devices()[:8]
    mesh = Mesh(np.asarray(devices), ("core",))
    sharding = NamedSharding(mesh, P("core"))
    in_specs = (P("core"),) * (n_params + n_outs)
    out_specs = (P("core"),) * n_outs
    sharded = jax.jit(
        _shard_map(_body, mesh, in_specs, out_specs),
        donate_argnums=donate,
        keep_unused=True,
    )
    zeros_fns = [
        jax.jit(
            lambda shape=shape, dtype=dtype: jnp.zeros((8 * shape[0],) + shape[1:], dtype),
            out_shardings=sharding,
        )
        for shape, dtype in zero_shapes
    ]
    runner = {
        "sharded": sharded,
        "in_names": in_names,
        "out_names": out_names,
        "out_avals": out_avals,
        "zeros_fns": zeros_fns,
        "sharding": sharding,
        "device_put": jax.device_put,
    }
    _cache["runner"] = runner
    return runner


def _weights_device(runner, w_qkv, w_dw, b_dw, w_proj, gamma, beta, temperature):
    """Upload per-core weight arrays once; reuse across calls when unchanged."""
    key = "weights"
    raw = (w_qkv, w_dw, b_dw, w_proj, gamma, beta, temperature)
    if key in _cache:
        saved_raw, dev = _cache[key]
        if all(np.array_equal(a, b) for a, b in zip(saved_raw, raw)):
            return dev
    wq_l, dw_l, wp_l, gb_l, tp_l, id_l = [], [], [], [], [], []
    eye = np.eye(D, dtype=np.float32)
    gb = np.stack([gamma, beta], axis=1).astype(np.float32)  # [C,2]
    temp = temperature.reshape(HEADS)
    taps9 = [(dy + 1) * 3 + (dx + 1) for (dy, dx) in TAPS]  # tap order -> w_dw idx
    for h in range(HEADS):
        sl = slice(h * D, (h + 1) * D)
        wq = np.zeros((C, 27 * D), np.float32)
        dw = np.zeros((D, 3), np.float32)
        for s, base in enumerate((h * D, C + h * D, 2 * C + h * D)):
            wsec = w_qkv[base : base + D]  # [D, C]
            dtap = w_dw[base : base + D, 0].reshape(D, 9)  # [D, 9] (dy,dx) row-major
            for t, t9 in enumerate(taps9):
                colb = (s * 9 + t) * D
                wq[:, colb : colb + D] = (wsec * dtap[:, t9 : t9 + 1]).T
            dw[:, s] = b_dw[base : base + D]
        wq_l.append(wq)
        dw_l.append(dw)
        wp = np.zeros((D + 1, C), np.float32)
        wp[1:, :] = w_proj[:, sl].T
        wp_l.append(wp)
        gb_l.append(gb)
        tp_l.append(temp[h : h + 1].reshape(1, 1).astype(np.float32))
        id_l.append(eye)
    by_name = {
        "wq": np.concatenate(wq_l, axis=0),
        "dw": np.concatenate(dw_l, axis=0),
        "wp": np.concatenate(wp_l, axis=0),
        "gb": np.concatenate(gb_l, axis=0),
        "tp": np.concatenate(tp_l, axis=0),
        "id24": np.concatenate(id_l, axis=0),
    }
    dev = {k: runner["device_put"](v, runner["sharding"]) for k, v in by_name.items()}
    for v in dev.values():
        v.block_until_ready()
    saved_raw = tuple(np.array(a, copy=True) for a in raw)
    _cache[key] = (saved_raw, dev)
    return dev


def kernel(x, gamma, beta, w_qkv, w_dw, b_dw, w_proj, temperature):
    x = np.asarray(x, dtype=np.float32)
    gamma = np.asarray(gamma, np.float32)
    beta = np.asarray(beta, np.float32)
    w_qkv = np.asarray(w_qkv, np.float32)
    w_dw = np.asarray(w_dw, np.float32)
    b_dw = np.asarray(b_dw, np.float32)
    w_proj = np.asarray(w_proj, np.float32)
    temperature = np.asarray(temperature, np.float32)

    runner = _make_runner()
    dev = _weights_device(runner, w_qkv, w_dw, b_dw, w_proj, gamma, beta, temperature)

    # shard x by image-row blocks: core c gets rows 8c..8c+8 -> [192, 512]
    x16 = x.reshape(C, HEADS, NP).astype(np.float16)
    xs = np.ascontiguousarray(x16.transpose(1, 0, 2)).reshape(8 * C, NP)

    args = []
    for name in runner["in_names"]:
        args.append(xs if name == "x" else dev[name])
    for zf in runner["zeros_fns"]:
        args.append(zf())
    outs = runner["sharded"](*args)
    y16 = np.asarray(outs[0])  # [8*192, 512] fp16; core c holds pixels c*512..
    y = y16.reshape(HEADS, C, NP).transpose(1, 0, 2).reshape(1, C, 64, 64)
    return y.astype(np.float32)
